# revision 72
# baseline (speedup 1.0000x reference)
"""SANet-style attention (nn_Attention_1382979470038) on 8 TRN2 NeuronCores.

Sharding: 8 cores = 4 batches x 2 content-token halves (sequence parallel on
N, style tokens replicated within each pair).  No collectives: each core
computes output columns [C=512, N_loc=2048] of its batch independently.

Math folding (host side, like the weight/bias folding the task allows):
  S[n,m] = F[:,n]^T G[:,m] with F = f_w x_hat + f_b, G = g_w s_hat + g_b.
  Under softmax over m, all terms constant in m drop (g_b, style-mean), so
    S = (ws2^T x + b2)^T s_raw,  ws2 = diag(rstd_c) (f_w^T g_w) diag(rstd_s)
    b2 = rstd_s . (g_w^T f_b - ws2'^T mean_c)
  where the instance-norm scalars (mean/rstd per channel, 8.4 MFLOP total)
  are folded into ws2/b2 on the host; the convs and the O(N M C) attention
  (99.7% of FLOPs) run on device.

Per-core device math (M = 4096 style tokens, N_loc = 2048 content tokens):
  F2 = ws2^T @ x_half + b2                       [C, N]
  St = style^T F2                                [M, N]  (m on partitions)
  P  = exp(St)            (no max-subtraction: |S| <~ 30 is fp32-safe)
  den[n] = sum_m P[m,n]   (vector/gpsimd add tree + one broadcast matmul)
  sraw = (styleT^T @ P) . (1/den)                [C, N]  (styleT host-sent)
  out = (out_w h_w) @ sraw + (out_b + out_w h_b)

The apply contracts raw style^T directly (host ships a pre-transposed
copy), so no on-device Ht conv is needed; the small (out_w h_w) conv runs
per chunk after the apply.  fp16 matmuls throughout; bf16 for the post-exp
apply.  Inputs are host-rearranged to k-major [128, ...] layouts so each
logical load is a single DMA.
"""

import sys

sys.path.insert(0, "/opt/trn_rl_repo")

import numpy as np

import concourse.bass as bass
import concourse.tile as tile
from concourse import mybir

P = 128
C = 512
HW = 4096
NLOC = 2048
EPS = 1e-5
KT = C // P        # 4 k-tiles of 128 channels
NCH = NLOC // 512  # 4 n-chunks of 512
MCH = HW // 512    # 8 m-chunks of 512
MT = HW // P       # 32 m-tiles of 128

F32 = mybir.dt.float32
BF16 = mybir.dt.bfloat16
F16 = mybir.dt.float16

AF = mybir.ActivationFunctionType
ALU = mybir.AluOpType


def build_nc(hoist=True):
    nc = bass.Bass()
    # k-major layouts: [128, KT * cols]; column block k holds rows k*128..
    cAp = nc.declare_dram_parameter("cA", [P, KT * NLOC], F16, isOutput=False)
    wsp_ = nc.declare_dram_parameter("ws2K", [P, KT * C], F16, isOutput=False)
    pbp = nc.declare_dram_parameter("pbias", [P, 2 * KT], F32, isOutput=False)
    sbfp = nc.declare_dram_parameter("style_bf", [P, KT * HW], F16,
                                     isOutput=False)
    stfp = nc.declare_dram_parameter("styleT", [P, MT * C], F16,
                                     isOutput=False)
    hwbfp = nc.declare_dram_parameter("hw_bf", [P, KT * C], F16,
                                      isOutput=False)
    out = nc.declare_dram_parameter("out", [C, NLOC], F32, isOutput=True)

    with tile.TileContext(nc) as tc:
        _build(tc, cAp, wsp_, pbp, sbfp, stfp, hwbfp, out)
    if hoist:
        _hoist_excess_waits(nc)
    return nc


# Walrus caps sync-wait commands per instruction (Activation/TensorScalar fit
# only one).  Hoist excess waits onto injected same-engine NOPs placed just
# before the instruction: engines execute in order, so semantics match.
def _hoist_excess_waits(nc):
    ctr = [0]

    def mknop(engine, debug, waits, updates):
        ctr[0] += 1
        return mybir.InstNoOp(
            name=f"WH-{ctr[0]}", opcode="NoOp", engine=engine, debug=debug,
            ins=[], outs=[],
            sync_info=mybir.SyncInfo(on_wait=waits, on_update=updates),
        )

    for fn in nc.m.functions:
        for blk in fn.blocks:
            newl = []
            changed = False
            for inst in blk.instructions:
                si = getattr(inst, "sync_info", None)
                if si is not None and si.on_wait and len(si.on_wait) > 1:
                    waits = list(si.on_wait)
                    keep, hoist = waits[-1:], waits[:-1]
                    eng = getattr(inst, "engine", None)
                    for w in hoist:
                        newl.append(mknop(eng, inst.debug, [w], []))
                    inst.sync_info = mybir.SyncInfo(
                        on_wait=keep, on_update=list(si.on_update))
                    changed = True
                newl.append(inst)
            if changed:
                blk.instructions = newl


def _build(tc, cAp, wsKp, pbp, sbfp, stfp, hwbfp, out):
    nc = tc.nc
    from contextlib import ExitStack

    ctx = ExitStack()
    with ctx:
        # ---------- pools ----------
        small = ctx.enter_context(tc.tile_pool(name="small", bufs=16))
        cons = ctx.enter_context(tc.tile_pool(name="cons", bufs=1))
        f2pool = ctx.enter_context(tc.tile_pool(name="f2pool", bufs=4))
        stypool = ctx.enter_context(tc.tile_pool(name="stypool", bufs=1))
        htpool = ctx.enter_context(tc.tile_pool(name="htpool", bufs=1))
        capool = ctx.enter_context(tc.tile_pool(name="capool", bufs=1))
        wspool = ctx.enter_context(tc.tile_pool(name="wspool", bufs=1))
        hwpool = ctx.enter_context(tc.tile_pool(name="hwpool", bufs=1))
        # PSUM pools (8 banks: 4 + 2 + 2)
        stps = ctx.enter_context(tc.tile_pool(name="stps", bufs=3,
                                              space="PSUM"))
        mmps = ctx.enter_context(tc.tile_pool(name="mmps", bufs=3,
                                              space="PSUM"))
        denps = ctx.enter_context(tc.tile_pool(name="denps", bufs=2,
                                               space="PSUM"))

        sT_sb = htpool.tile([P, MT * C], F16, tag="sT")
        style_sb = stypool.tile([P, KT * HW], F16, tag="style")
        hw_bf = hwpool.tile([P, KT * C], F16, tag="hwbf")
        cA_s = capool.tile([P, KT * NLOC], F16, tag="cA")
        ws_sb = wspool.tile([P, KT * C], F16, tag="ws")

        # ================= DMA dispatch (sync-queue FIFO order) ===========
        # F2 deps (cA/ws2/pbias) first -- they gate the very first matmul;
        # style chunks pace the scores; styleT/hw_bf only gate the apply.
        def ca_dma(ch):
            nc.sync.dma_start(cA_s[:, ch * KT * 512:(ch + 1) * KT * 512],
                              cAp[:, ch * KT * 512:(ch + 1) * KT * 512])

        def style_dma(ch):
            nc.sync.dma_start(
                style_sb[:, ch * KT * 512:(ch + 1) * KT * 512],
                sbfp[:, ch * KT * 512:(ch + 1) * KT * 512])

        nc.sync.dma_start(cA_s[:, 0:512], cAp[:, 0:512])
        nc.sync.dma_start(ws_sb[:, 0:C], wsKp[:, 0:C])
        nc.sync.dma_start(cA_s[:, 512:2 * 512], cAp[:, 512:2 * 512])
        nc.sync.dma_start(ws_sb[:, C:2 * C], wsKp[:, C:2 * C])
        nc.sync.dma_start(cA_s[:, 2 * 512:KT * 512],
                          cAp[:, 2 * 512:KT * 512])
        nc.sync.dma_start(ws_sb[:, 2 * C:KT * C], wsKp[:, 2 * C:KT * C])
        pb_sb = cons.tile([P, 2 * KT], F32, tag="pb_sb")
        nc.sync.dma_start(pb_sb[:], pbp[:, :])
        style_dma(0)
        style_dma(1)
        ca_dma(1)
        style_dma(2)
        ca_dma(2)
        style_dma(3)
        ca_dma(3)
        for ch in range(4, MCH):
            style_dma(ch)
        for q in range(4):
            nc.sync.dma_start(
                sT_sb[:, q * 8 * C:(q + 1) * 8 * C],
                stfp[:, q * 8 * C:(q + 1) * 8 * C])
        nc.sync.dma_start(hw_bf[:], hwbfp[:, :])
        ones_bf = cons.tile([P, P], BF16, tag="ones_bf")
        nc.vector.memset(ones_bf[:], 1.0)

        # ============== attention: F2 -> scores -> apply ==============
        F2_sb = [f2pool.tile([P, NLOC], F16, tag="F2", name=f"F2{k}")
                 for k in range(KT)]

        def emit_f2(ch):
            # F2[c',n] = sum_i ws2[i,c'] cA[i,n] + b2[c']
            for j in range(KT):
                ps = stps.tile([P, 512], F32, tag="stps")
                for k in range(KT):
                    nc.tensor.matmul(
                        ps[:],
                        ws_sb[:, k * C + j * P: k * C + (j + 1) * P],
                        cA_s[:, ch * KT * 512 + k * 512:
                             ch * KT * 512 + (k + 1) * 512],
                        start=(k == 0), stop=(k == KT - 1))
                nc.scalar.activation(
                    F2_sb[j][:, ch * 512:(ch + 1) * 512], ps[:],
                    AF.Identity, bias=pb_sb[:, j: j + 1])

        emit_f2(0)
        with tc.tile_pool(name="expp", bufs=2) as expp, \
             tc.tile_pool(name="denp", bufs=5) as denp, \
             tc.tile_pool(name="styp", bufs=4) as styp, \
             tc.tile_pool(name="srawp", bufs=5) as srawp, \
             tc.tile_pool(name="rdenp", bufs=2) as rdenp:
            for ch in range(NCH):
                # ---- scores + exp ----
                exp_t = expp.tile([P, MT * 512], BF16, tag="exp")
                for mt in range(MT):
                    sch, mi = mt // 4, mt % 4
                    ps = stps.tile([P, 512], F32, tag="stps")
                    for k in range(KT):
                        nc.tensor.matmul(
                            ps[:],
                            style_sb[:, sch * KT * 512 + k * 512 + mi * P:
                                     sch * KT * 512 + k * 512
                                     + (mi + 1) * P],
                            F2_sb[k][:, ch * 512:(ch + 1) * 512],
                            start=(k == 0), stop=(k == KT - 1))
                    nc.scalar.activation(
                        exp_t[:, mt * 512:(mt + 1) * 512], ps[:], AF.Exp)
                if ch + 1 < NCH:
                    emit_f2(ch + 1)

                # ---- den: add tree over the 32 exp tiles -------------
                # pairs split vector/gpsimd; f32 accumulation per engine
                def esl(mt):
                    return exp_t[:, mt * 512:(mt + 1) * 512]

                accs = []
                for eng, base, npair in ((nc.vector, 0, 10),
                                         (nc.gpsimd, 20, 6)):
                    acc = None
                    for i in range(npair):
                        t = denp.tile([P, 512], F32, tag="den")
                        eng.tensor_add(t[:], esl(base + 2 * i),
                                       esl(base + 2 * i + 1))
                        if acc is None:
                            acc = t
                        else:
                            a2 = denp.tile([P, 512], F32, tag="den")
                            eng.tensor_add(a2[:], acc[:], t[:])
                            acc = a2
                    accs.append(acc)
                den_bf = denp.tile([P, 512], BF16, tag="denbf", bufs=2)
                nc.vector.tensor_add(den_bf[:], accs[0][:], accs[1][:])

                # ---- apply: sraw_j = sT_j^T @ P, . 1/den (fp16) ------
                rden = rdenp.tile([P, 512], F32, tag="rden")
                sraw = []
                for j in range(KT):
                    ps = mmps.tile([P, 512], F32, tag="mmps")
                    for mt in range(MT):
                        nc.tensor.matmul(
                            ps[:],
                            sT_sb[:, mt * C + j * P: mt * C + (j + 1) * P],
                            exp_t[:, mt * 512:(mt + 1) * 512],
                            start=(mt == 0), stop=(mt == MT - 1))
                    if j == 0:
                        # broadcast den over partitions via ones matmul
                        dps = denps.tile([P, 512], F32, tag="denps")
                        nc.tensor.matmul(dps[:], ones_bf[:], den_bf[:],
                                         start=True, stop=True)
                        nc.vector.reciprocal(rden[:], dps[:])
                    st = srawp.tile([P, 512], F16, tag="sraw")
                    nc.vector.tensor_mul(st[:], ps[:], rden[:])
                    sraw.append(st)

                # ---- out conv: out2 = (out_w h_w) @ sraw + bias ------
                for j in range(KT):
                    ps = stps.tile([P, 512], F32, tag="stps")
                    for k in range(KT):
                        nc.tensor.matmul(
                            ps[:],
                            hw_bf[:, k * C + j * P: k * C + (j + 1) * P],
                            sraw[k][:],
                            start=(k == 0), stop=(k == KT - 1))
                    s_t = styp.tile([P, 512], F32, tag="sty")
                    if ch == NCH - 1 and j == KT - 1:
                        # final tile: halves biased on scalar and vector in
                        # parallel so the end-of-kernel chain is half as long
                        sl = slice(0, 256)
                        nc.scalar.activation(
                            s_t[:, sl], ps[:, sl], AF.Identity,
                            bias=pb_sb[:, KT + j: KT + j + 1])
                        sl2 = slice(256, 512)
                        nc.vector.tensor_scalar(
                            s_t[:, sl2], ps[:, sl2],
                            pb_sb[:, KT + j: KT + j + 1], None, op0=ALU.add)
                        nc.sync.dma_start(
                            out[j * P:(j + 1) * P,
                                ch * 512:ch * 512 + 256], s_t[:, 0:256])
                        nc.sync.dma_start(
                            out[j * P:(j + 1) * P,
                                ch * 512 + 256:(ch + 1) * 512],
                            s_t[:, 256:512])
                    else:
                        nc.scalar.activation(
                            s_t[:], ps[:], AF.Identity,
                            bias=pb_sb[:, KT + j: KT + j + 1])
                        nc.sync.dma_start(
                            out[j * P:(j + 1) * P, ch * 512:(ch + 1) * 512],
                            s_t[:])


def _kmajor(x, cols):
    """[KT*128, cols] -> [128, KT*cols] with column block k = rows k*128.."""
    return np.ascontiguousarray(
        np.asarray(x).reshape(KT, P, cols).transpose(1, 0, 2)
        .reshape(P, KT * cols), dtype=np.float32)


_NC_CACHE = None


def _get_nc():
    global _NC_CACHE
    if _NC_CACHE is None:
        _NC_CACHE = build_nc()
    return _NC_CACHE


def make_in_maps(content, style, f_w, f_b, g_w, g_b, h_w, h_b, out_w, out_b):
    b, Cc, H, W = content.shape
    hw = H * W
    cf = np.ascontiguousarray(content.reshape(b, Cc, hw), dtype=np.float32)
    sf = np.ascontiguousarray(style.reshape(b, Cc, hw), dtype=np.float32)
    # host-folded scalars: instance-norm stats per (batch, channel)
    cf64 = cf.astype(np.float64)
    sf64 = sf.astype(np.float64)
    cmean = cf64.mean(axis=2)                                   # [b, C]
    crstd = 1.0 / np.sqrt(cf64.var(axis=2, ddof=1) + EPS)       # [b, C]
    srstd = 1.0 / np.sqrt(sf64.var(axis=2, ddof=1) + EPS)       # [b, C]
    ob2 = (np.asarray(out_b, np.float64)
           + np.asarray(out_w, np.float64) @ np.asarray(h_b, np.float64))
    gfb = np.asarray(g_w, np.float64).T @ np.asarray(f_b, np.float64)
    hw2 = np.asarray(out_w, np.float64) @ np.asarray(h_w, np.float64)
    wu = np.asarray(f_w, np.float64).T @ np.asarray(g_w, np.float64)
    wT = {
        "hw_bf": _kmajor(hw2.T.astype(np.float32), C).astype(np.float16),
    }
    in_maps = []
    per_batch = []
    for bi in range(b):
        wsb = wu * crstd[bi][:, None]                    # [c_in, c']
        b2 = srstd[bi] * (gfb - wsb.T @ cmean[bi])       # [c']
        ws2 = wsb * srstd[bi][None, :]
        pbias = np.concatenate([
            b2.astype(np.float32).reshape(KT, P).T,
            ob2.astype(np.float32).reshape(KT, P).T], axis=1)
        sT = np.ascontiguousarray(
            sf[bi].T.reshape(MT, P, C).transpose(1, 0, 2)
            .reshape(P, MT * C), dtype=np.float16)
        per_batch.append({
            "ws2K": _kmajor(ws2.astype(np.float32), C).astype(np.float16),
            "pbias": np.ascontiguousarray(pbias, dtype=np.float32),
            "style_bf": np.concatenate(
                [_kmajor(sf[bi][:, ch * 512:(ch + 1) * 512], 512)
                 for ch in range(MCH)], axis=1).astype(np.float16),
            "styleT": sT,
        })
    for core in range(8):
        bi, hi = core // 2, core % 2
        half = cf[bi][:, hi * NLOC:(hi + 1) * NLOC]
        in_maps.append({
            "cA": np.concatenate(
                [_kmajor(half[:, ch * 512:(ch + 1) * 512], 512)
                 for ch in range(NCH)], axis=1).astype(np.float16),
            **per_batch[bi],
            **wT,
        })
    return in_maps


def kernel(content, style, f_w, f_b, g_w, g_b, h_w, h_b, out_w, out_b):
    from concourse.bass_utils import run_bass_kernel_spmd

    global _LAST_IN_MAPS
    in_maps = make_in_maps(content, style, f_w, f_b, g_w, g_b, h_w, h_b,
                           out_w, out_b)
    _LAST_IN_MAPS = in_maps
    b, Cc, H, W = content.shape
    hw = H * W
    nc = _get_nc()
    res = run_bass_kernel_spmd(nc, in_maps, core_ids=list(range(8)))
    outf = np.empty((b, Cc, hw), dtype=np.float32)
    for core in range(8):
        bi, hi = core // 2, core % 2
        outf[bi][:, hi * NLOC:(hi + 1) * NLOC] = res.results[core]["out"]
    return outf.reshape(b, Cc, H, W)
